# revision 6
# baseline (speedup 1.0000x reference)
"""Depth-modulated 3x3 conv (DepthConv) for Trainium2, 8-way batch-parallel.

out(b,o,h,w) = sum_{c,i,j} W[o,c,i,j] * x[b,c,h+i-1,w+j-1]
               * exp(-8.3*|d[b,h,w] - d[b,h+i-1,w+j-1]|)

Strategy (per core = one batch item):
  - Pixels are matmul OUTPUT partitions: 32 tiles of 128 px (2 rows).
  - For each row-shift i in {0,1,2}: stationary lhsT = x[cin_chunk, 128 px
    shifted by (i-1)*64] (bf16), moving rhs = W[cin_chunk, (j,o)=192] (bf16),
    4 cin chunks accumulate in PSUM -> y3_i[p, (j,o)].
  - The w-shift (j-1 = +-1) becomes a +-1 PSUM partition shift, done by DMA
    into SBUF staging (engines can't shift partition bases; DMA can).
  - Gate g_ij[p] enters as a per-partition scalar in fused DVE
    scalar_tensor_tensor ops: acc = (y * g) + acc.  All out-of-bounds /
    wrap-around garbage is killed by gates that are exactly 0 (host bakes
    d_shift = d0 + 100 at invalid taps -> exp underflows to 0).
  - Gates are computed on device from host-relayouted shifted depth.
Output is written [4096 px, 64 cout] contiguous; host transposes back.
"""
import os
import sys
sys.path.insert(0, '/opt/trn_rl_repo')

import numpy as np
import ml_dtypes

import concourse.bass as bass
import concourse.tile as tile
from concourse import bacc, mybir
from concourse.bass_utils import run_bass_kernel_spmd

F32 = mybir.dt.float32
BF16 = mybir.dt.bfloat16

B, CIN, H, W = 8, 512, 64, 64
COUT, K = 64, 3
ALPHA = 8.3
NPX = H * W            # 4096
NT = NPX // 128        # 32 pixel tiles
KC = CIN // 128        # 4 cin chunks
XCOLS = NPX + 128      # 64 guard + 4096 + 64 guard

_cache = {}


def build_nc():
    nc = bacc.Bacc("TRN2", target_bir_lowering=False, debug=False, num_devices=B)
    x_d = nc.dram_tensor("x", [128, KC, XCOLS], BF16, kind="ExternalInput").ap()
    w_d = nc.dram_tensor("w", [128, KC, 3, 192], BF16, kind="ExternalInput").ap()
    dsh_d = nc.dram_tensor("dsh", [128, 9, 32], F32, kind="ExternalInput").ap()
    out_d = nc.dram_tensor("out", [NPX, COUT], F32, kind="ExternalOutput").ap()

    with tile.TileContext(nc) as tc:
        with tc.tile_pool(name="const", bufs=1) as cpool, \
             tc.tile_pool(name="work", bufs=3) as wpool, \
             tc.tile_pool(name="accp", bufs=4) as apool, \
             tc.tile_pool(name="psum", bufs=2, space="PSUM") as ppool:

            x_sb = cpool.tile([128, KC, XCOLS], BF16)
            for k in range(KC):
                nc.sync.dma_start(x_sb[:, k, :], x_d[:, k, :])
            w_sb = cpool.tile([128, KC, 3, 192], BF16)
            nc.sync.dma_start(w_sb[:], w_d[:])
            dsh = cpool.tile([128, 9, 32], F32)
            nc.sync.dma_start(dsh[:], dsh_d[:])

            # gates: g_all[:, ij, t] = exp(-8.3 * |dsh_ij - d0|)
            diff = cpool.tile([128, 9, 32], F32)
            for ij in range(9):
                nc.vector.tensor_tensor(diff[:, ij, :], dsh[:, ij, :], dsh[:, 4, :],
                                        op=mybir.AluOpType.subtract)
            nc.scalar.activation(diff[:, :, :], diff[:, :, :],
                                 mybir.ActivationFunctionType.Abs)
            g_all = cpool.tile([128, 9, 32], F32)
            nc.scalar.activation(g_all[:, :, :], diff[:, :, :],
                                 mybir.ActivationFunctionType.Exp, scale=-ALPHA)
            # center gate is exactly 1 (no LUT roundoff)
            nc.vector.memset(g_all[:, 4, :], 1.0)

            for t in range(NT):
                ps = ppool.tile([128, 4, 256], F32)
                for i in range(3):
                    base = 64 + t * 128 + (i - 1) * 64
                    for k in range(KC):
                        nc.tensor.matmul(
                            ps[:, i, 0:192],
                            x_sb[:, k, base:base + 128],
                            w_sb[:, k, i, :],
                            start=(k == 0), stop=(k == KC - 1),
                        )

                # PSUM -> SBUF (DMA can't read PSUM; engines can't shift
                # partitions) then DMA does the +-1 partition shifts.
                y_sb = wpool.tile([128, 3, 192], F32)
                nc.scalar.copy(y_sb[:], ps[:, 0:3, 0:192])
                # j=0 blocks, shifted +1 partition (out[p] = y[p-1])
                ysh0 = wpool.tile([128, 3, 64], F32)
                nc.sync.dma_start(ysh0[1:128, :, :], y_sb[0:127, :, 0:64])
                nc.sync.dma_start(ysh0[0:1, :, :], y_sb[0:1, :, 0:64])
                # j=2 blocks, shifted -1 partition (out[p] = y[p+1])
                ysh2 = wpool.tile([128, 3, 64], F32)
                nc.sync.dma_start(ysh2[0:127, :, :], y_sb[1:128, :, 128:192])
                nc.sync.dma_start(ysh2[127:128, :, :], y_sb[127:128, :, 128:192])

                # center term (gate == 1) seeds the accumulator
                acc = apool.tile([128, COUT], F32)
                nc.scalar.copy(acc[:], ps[:, 1, 64:128])

                for i in range(3):
                    for j in range(3):
                        if i == 1 and j == 1:
                            continue
                        if j == 0:
                            src = ysh0[:, i, :]
                        elif j == 2:
                            src = ysh2[:, i, :]
                        else:
                            src = y_sb[:, i, 64:128]
                        g = g_all[:, 3 * i + j, t:t + 1]
                        nc.vector.scalar_tensor_tensor(
                            acc[:], src, g, acc[:],
                            op0=mybir.AluOpType.mult, op1=mybir.AluOpType.add)

                nc.sync.dma_start(out_d[t * 128:(t + 1) * 128, :], acc[:])

    nc.compile()
    return nc


def prep_inputs(input, depth, weight):
    """Host-side relayout: returns per-core in_maps."""
    # x: (B, 512, 64, 64) -> [128, KC, XCOLS] bf16 with zero guards
    xr = input.reshape(B, KC, 128, NPX).transpose(0, 2, 1, 3)  # [B,128,KC,NPX]
    x_all = np.zeros((B, 128, KC, XCOLS), dtype=ml_dtypes.bfloat16)
    x_all[:, :, :, 64:64 + NPX] = xr.astype(ml_dtypes.bfloat16)

    # w: (64, 512, 3, 3) -> [128, KC, 3(i), 192(j*64+o)] bf16
    wr = weight.reshape(COUT, KC, 128, 3, 3)
    w_dev = wr.transpose(2, 1, 3, 4, 0).reshape(128, KC, 3, 192)
    w_dev = np.ascontiguousarray(w_dev).astype(ml_dtypes.bfloat16)

    # dsh: shifted depth with +100 poison at invalid taps, in [p, ij, t] layout
    d = depth.reshape(B, H, W).astype(np.float32)
    dsh_all = np.empty((B, 128, 9, 32), dtype=np.float32)
    for i in range(3):
        for j in range(3):
            sh = np.full((B, H, W), 0.0, np.float32)
            hs0, hs1 = max(0, 1 - i), min(H, H + 1 - i)
            ws0, ws1 = max(0, 1 - j), min(W, W + 1 - j)
            sh[:, hs0:hs1, ws0:ws1] = d[:, hs0 + i - 1:hs1 + i - 1,
                                        ws0 + j - 1:ws1 + j - 1]
            invalid = np.ones((H, W), dtype=bool)
            invalid[hs0:hs1, ws0:ws1] = False
            sh[:, invalid] = d[:, invalid] + 100.0
            # (B,64,64) -> [B, p=(h&1)*64+w, t=h>>1]
            dsh_all[:, :, 3 * i + j, :] = (
                sh.reshape(B, 32, 2, W).transpose(0, 2, 3, 1).reshape(B, 128, 32))

    return [
        {"x": x_all[b], "w": w_dev, "dsh": dsh_all[b]}
        for b in range(B)
    ]


def kernel(input, depth, weight):
    input = np.asarray(input, dtype=np.float32)
    depth = np.asarray(depth, dtype=np.float32)
    weight = np.asarray(weight, dtype=np.float32)

    if "nc" not in _cache:
        _cache["nc"] = build_nc()
    nc = _cache["nc"]

    in_maps = prep_inputs(input, depth, weight)
    kwargs = {}
    if os.environ.get("KERNEL_TRACE") == "1":
        kwargs = dict(trace=True, trace_cores=list(range(B)))
    res = run_bass_kernel_spmd(nc, in_maps, core_ids=list(range(B)), **kwargs)
    _cache["last_results"] = res
    out = np.stack([
        res.results[b]["out"].T.reshape(COUT, H, W) for b in range(B)
    ]).astype(np.float32)
    return out


if __name__ == "__main__":
    rng = np.random.default_rng(0)
    x = rng.standard_normal((B, CIN, H, W), dtype=np.float32)
    d = rng.random((B, 1, H, W), dtype=np.float32)
    w = (rng.random((COUT, CIN, 3, 3), dtype=np.float32) - 0.5) * 0.08
    o = kernel(x, d, w)
    print(o.shape, o.dtype)


# revision 7
# speedup vs baseline: 1.4585x; 1.4585x over previous
"""Depth-modulated 3x3 conv (DepthConv) for Trainium2, 8-way batch-parallel.

out(b,o,h,w) = sum_{c,i,j} W[o,c,i,j] * x[b,c,h+i-1,w+j-1]
               * exp(-8.3*|d[b,h,w] - d[b,h+i-1,w+j-1]|)

Strategy (per core = one batch item):
  - Pixels are matmul OUTPUT partitions: 32 tiles of 128 px (2 rows).
  - For each row-shift i in {0,1,2}: stationary lhsT = x[cin_chunk, 128 px
    shifted by (i-1)*64] (bf16), moving rhs = W[cin_chunk, (j,o)=192] (bf16),
    4 cin chunks accumulate in PSUM -> y3_i[p, (j,o)].
  - The w-shift (j-1 = +-1) becomes a +-1 PSUM partition shift, which
    engines cannot do ({0,32,64,96} partition-base rule) but DMA can.
    Tiles are processed in groups of 8 so the shift is 2 large contiguous
    SBUF->SBUF DMAs per group instead of many tiny strided ones.
  - Gate g_ij[p] enters as a per-partition scalar in fused DVE
    scalar_tensor_tensor ops: acc = (y * g) + acc.  All out-of-bounds /
    wrap-around garbage is killed by gates that are exactly 0 (host bakes
    d_shift = d0 + 100 at invalid taps -> exp underflows to 0).
  - Gates are computed on device from host-relayouted shifted depth.
Output is written [128 p, 32 t, 64 o] contiguous; host re-layouts.
"""
import os
import sys
sys.path.insert(0, '/opt/trn_rl_repo')

import numpy as np
import ml_dtypes

import concourse.bass as bass
import concourse.tile as tile
from concourse import bacc, mybir
from concourse.bass_utils import run_bass_kernel_spmd

F32 = mybir.dt.float32
BF16 = mybir.dt.bfloat16

B, CIN, H, W = 8, 512, 64, 64
COUT, K = 64, 3
ALPHA = 8.3
NPX = H * W            # 4096
NT = NPX // 128        # 32 pixel tiles
KC = CIN // 128        # 4 cin chunks
XCOLS = NPX + 128      # 64 guard + 4096 + 64 guard
GT = 8                 # tiles per group
NG = NT // GT          # 4 groups

_cache = {}


def build_nc():
    nc = bacc.Bacc("TRN2", target_bir_lowering=False, debug=False, num_devices=B)
    x_d = nc.dram_tensor("x", [128, KC, XCOLS], BF16, kind="ExternalInput").ap()
    w_d = nc.dram_tensor("w", [128, KC, 3, 192], BF16, kind="ExternalInput").ap()
    dsh_d = nc.dram_tensor("dsh", [128, 9, 32], F32, kind="ExternalInput").ap()
    out_d = nc.dram_tensor("out", [128, NT, COUT], F32, kind="ExternalOutput").ap()

    with tile.TileContext(nc) as tc:
        with tc.tile_pool(name="const", bufs=1) as cpool, \
             tc.tile_pool(name="ygrp", bufs=2) as ypool, \
             tc.tile_pool(name="psum", bufs=4, space="PSUM") as ppool:

            x_sb = cpool.tile([128, KC, XCOLS], BF16)
            for k in range(KC):
                nc.gpsimd.dma_start(x_sb[:, k, :], x_d[:, k, :])
            w_sb = cpool.tile([128, KC, 3, 192], BF16)
            nc.gpsimd.dma_start(w_sb[:], w_d[:])
            dsh = cpool.tile([128, 9, 32], F32)
            nc.gpsimd.dma_start(dsh[:], dsh_d[:])

            # gates: g_all[:, ij, t] = exp(-8.3 * |dsh_ij - d0|)
            diff = cpool.tile([128, 9, 32], F32)
            for ij in range(9):
                nc.vector.tensor_tensor(diff[:, ij, :], dsh[:, ij, :], dsh[:, 4, :],
                                        op=mybir.AluOpType.subtract)
            nc.scalar.activation(diff[:, :, :], diff[:, :, :],
                                 mybir.ActivationFunctionType.Abs)
            g_all = cpool.tile([128, 9, 32], F32)
            nc.scalar.activation(g_all[:, :, :], diff[:, :, :],
                                 mybir.ActivationFunctionType.Exp, scale=-ALPHA)
            # center gate is exactly 1 (no LUT roundoff)
            nc.vector.memset(g_all[:, 4, :], 1.0)

            # persistent accumulator; one output DMA at the end
            acc_all = cpool.tile([128, NT, COUT], F32)

            for g in range(NG):
                # y_grp[p, j, tg, i, o]: j-major so the per-group shift DMA
                # reads/writes contiguous 6KB per partition
                y_grp = ypool.tile([128, 3, GT, 3, COUT], F32, tag="ygrp")
                for tg in range(GT):
                    t = g * GT + tg
                    ps = ppool.tile([128, 4, 256], F32, tag="ps")
                    for i in range(3):
                        base = 64 + t * 128 + (i - 1) * 64
                        for k in range(KC):
                            nc.tensor.matmul(
                                ps[:, i, 0:192],
                                x_sb[:, k, base:base + 128],
                                w_sb[:, k, i, :],
                                start=(k == 0), stop=(k == KC - 1),
                            )
                    # scatter psum [i, (j,o)] -> y_grp [j, tg, i, o]
                    nc.scalar.copy(
                        y_grp[:, :, tg, :, :],
                        ps[:, 0:3, 0:192].rearrange("p i (j o) -> p j i o", j=3))
                    # center term (gate == 1) seeds the accumulator
                    t_ = g * GT + tg
                    nc.scalar.copy(acc_all[:, t_, :], ps[:, 1, 64:128])

                # j=0 blocks shifted +1 partition (out[p] = y[p-1])
                ysh0 = ypool.tile([128, GT, 3, COUT], F32, tag="ysh0")
                nc.sync.dma_start(ysh0[1:128], y_grp[0:127, 0])
                nc.sync.dma_start(ysh0[0:1], y_grp[0:1, 0])
                # j=2 blocks shifted -1 partition (out[p] = y[p+1])
                ysh2 = ypool.tile([128, GT, 3, COUT], F32, tag="ysh2")
                nc.sync.dma_start(ysh2[0:127], y_grp[1:128, 2])
                nc.sync.dma_start(ysh2[127:128], y_grp[127:128, 2])

                for tg in range(GT):
                    t = g * GT + tg
                    for i in range(3):
                        for j in range(3):
                            if i == 1 and j == 1:
                                continue
                            if j == 0:
                                src = ysh0[:, tg, i, :]
                            elif j == 2:
                                src = ysh2[:, tg, i, :]
                            else:
                                src = y_grp[:, 1, tg, i, :]
                            gp = g_all[:, 3 * i + j, t:t + 1]
                            nc.vector.scalar_tensor_tensor(
                                acc_all[:, t, :], src, gp, acc_all[:, t, :],
                                op0=mybir.AluOpType.mult, op1=mybir.AluOpType.add)

            nc.sync.dma_start(out_d[:], acc_all[:])

    nc.compile()
    return nc


def prep_inputs(input, depth, weight):
    """Host-side relayout: returns per-core in_maps."""
    # x: (B, 512, 64, 64) -> [128, KC, XCOLS] bf16 with zero guards
    xr = input.reshape(B, KC, 128, NPX).transpose(0, 2, 1, 3)  # [B,128,KC,NPX]
    x_all = np.zeros((B, 128, KC, XCOLS), dtype=ml_dtypes.bfloat16)
    x_all[:, :, :, 64:64 + NPX] = xr.astype(ml_dtypes.bfloat16)

    # w: (64, 512, 3, 3) -> [128, KC, 3(i), 192(j*64+o)] bf16
    wr = weight.reshape(COUT, KC, 128, 3, 3)
    w_dev = wr.transpose(2, 1, 3, 4, 0).reshape(128, KC, 3, 192)
    w_dev = np.ascontiguousarray(w_dev).astype(ml_dtypes.bfloat16)

    # dsh: shifted depth with +100 poison at invalid taps, in [p, ij, t] layout
    d = depth.reshape(B, H, W).astype(np.float32)
    dsh_all = np.empty((B, 128, 9, 32), dtype=np.float32)
    for i in range(3):
        for j in range(3):
            sh = np.full((B, H, W), 0.0, np.float32)
            hs0, hs1 = max(0, 1 - i), min(H, H + 1 - i)
            ws0, ws1 = max(0, 1 - j), min(W, W + 1 - j)
            sh[:, hs0:hs1, ws0:ws1] = d[:, hs0 + i - 1:hs1 + i - 1,
                                        ws0 + j - 1:ws1 + j - 1]
            invalid = np.ones((H, W), dtype=bool)
            invalid[hs0:hs1, ws0:ws1] = False
            sh[:, invalid] = d[:, invalid] + 100.0
            # (B,64,64) -> [B, p=(h&1)*64+w, t=h>>1]
            dsh_all[:, :, 3 * i + j, :] = (
                sh.reshape(B, 32, 2, W).transpose(0, 2, 3, 1).reshape(B, 128, 32))

    return [
        {"x": x_all[b], "w": w_dev, "dsh": dsh_all[b]}
        for b in range(B)
    ]


def kernel(input, depth, weight):
    input = np.asarray(input, dtype=np.float32)
    depth = np.asarray(depth, dtype=np.float32)
    weight = np.asarray(weight, dtype=np.float32)

    if "nc" not in _cache:
        _cache["nc"] = build_nc()
    nc = _cache["nc"]

    in_maps = prep_inputs(input, depth, weight)
    kwargs = {}
    if os.environ.get("KERNEL_TRACE") == "1":
        kwargs = dict(trace=True, trace_cores=list(range(B)))
    res = run_bass_kernel_spmd(nc, in_maps, core_ids=list(range(B)), **kwargs)
    _cache["last_results"] = res
    # out_dev [128 p, 32 t, 64 o] -> (COUT, H, W): px = 128*t + p
    out = np.stack([
        res.results[b]["out"].transpose(1, 0, 2).reshape(NPX, COUT)
        .T.reshape(COUT, H, W)
        for b in range(B)
    ]).astype(np.float32)
    return out


if __name__ == "__main__":
    rng = np.random.default_rng(0)
    x = rng.standard_normal((B, CIN, H, W), dtype=np.float32)
    d = rng.random((B, 1, H, W), dtype=np.float32)
    w = (rng.random((COUT, CIN, 3, 3), dtype=np.float32) - 0.5) * 0.08
    o = kernel(x, d, w)
    print(o.shape, o.dtype)


# revision 8
# speedup vs baseline: 4.2137x; 2.8890x over previous
"""Depth-modulated 3x3 conv (DepthConv) for Trainium2, 8-way batch-parallel.

out(b,o,h,w) = sum_{c,i,j} W[o,c,i,j] * x[b,c,h+i-1,w+j-1]
               * exp(-8.3*|d[b,h,w] - d[b,h+i-1,w+j-1]|)

Strategy (per core = one batch item):
  - Pixels are matmul OUTPUT partitions: 32 tiles of 128 px (2 rows).
  - For each row-shift i in {0,1,2}: stationary lhsT = x[cin_chunk, 128 px
    shifted by (i-1)*64] (bf16), moving rhs = W[cin_chunk, (j,o)=192] (bf16),
    4 cin chunks accumulate in PSUM -> y3[p, i, (j,o)].
  - Gating: one DVE tensor_tensor multiplies the whole PSUM tile by the
    9 gates (per-partition, broadcast along cout via step-0 AP), using
    gates pre-shifted by (1-j) so every operand is partition-aligned.
    One DVE tensor_reduce sums over i, keeping (j, cout).
  - The leftover w-shift (j-1 = +-1) and the sum over j happen in the
    OUTPUT DMA: three SWDGE DMAs per tile-group accumulate (accum_op=add)
    into a zero-initialized DRAM buffer at row offsets 2-j.  DRAM rows
    have no partition-alignment constraints, so the shift is free there.
  - All out-of-bounds / wrap-around garbage is killed by gates that are
    exactly 0 (host bakes d_shift = d_center + 100 at invalid taps ->
    exp underflows to 0).
Output rows 1..4097 of [4098, 64] hold pixel-major results; host re-layouts.
"""
import os
import sys
sys.path.insert(0, '/opt/trn_rl_repo')

import numpy as np
import ml_dtypes

import concourse.bass as bass
import concourse.tile as tile
from concourse import bacc, mybir
from concourse.bass_utils import run_bass_kernel_spmd

F32 = mybir.dt.float32
BF16 = mybir.dt.bfloat16

B, CIN, H, W = 8, 512, 64, 64
COUT, K = 64, 3
ALPHA = 8.3
NPX = H * W            # 4096
NT = NPX // 128        # 32 pixel tiles
KC = CIN // 128        # 4 cin chunks
XCOLS = NPX + 128      # 64 guard + 4096 + 64 guard
GT = 8                 # tiles per group
NG = NT // GT          # 4 groups
OROWS = NPX + 2        # guard row at 0 and 4097

_cache = {}


def build_nc():
    nc = bacc.Bacc("TRN2", target_bir_lowering=False, debug=False, num_devices=B)
    x_d = nc.dram_tensor("x", [128, KC, XCOLS], BF16, kind="ExternalInput").ap()
    w_d = nc.dram_tensor("w", [128, KC, 3, 192], BF16, kind="ExternalInput").ap()
    dsh0_d = nc.dram_tensor("dsh0", [128, 9, 32], F32, kind="ExternalInput").ap()
    dsh1_d = nc.dram_tensor("dsh1", [128, 9, 32], F32, kind="ExternalInput").ap()
    out_d = nc.dram_tensor("out", [OROWS, COUT], F32, kind="ExternalOutput").ap()

    with tile.TileContext(nc) as tc:
        with tc.tile_pool(name="const", bufs=1) as cpool, \
             tc.tile_pool(name="work", bufs=3) as wpool, \
             tc.tile_pool(name="pgrp", bufs=2) as gpool, \
             tc.tile_pool(name="psum", bufs=4, space="PSUM") as ppool:

            x_sb = cpool.tile([128, KC, XCOLS], BF16)
            for k in range(KC):
                nc.sync.dma_start(x_sb[:, k, :], x_d[:, k, :])
            w_sb = cpool.tile([128, KC, 3, 192], BF16)
            nc.sync.dma_start(w_sb[:], w_d[:])
            dsh0 = cpool.tile([128, 9, 32], F32)
            nc.sync.dma_start(dsh0[:], dsh0_d[:])
            dsh1 = cpool.tile([128, 9, 32], F32)
            nc.sync.dma_start(dsh1[:], dsh1_d[:])

            # gates: g_all[:, ij, t] = exp(-8.3 * |dsh1 - dsh0|), center == 1
            diff = cpool.tile([128, 9, 32], F32)
            nc.vector.tensor_tensor(diff[:], dsh1[:], dsh0[:],
                                    op=mybir.AluOpType.subtract)
            nc.scalar.activation(diff[:], diff[:],
                                 mybir.ActivationFunctionType.Abs)
            g_all = cpool.tile([128, 9, 32], F32)
            nc.scalar.activation(g_all[:], diff[:],
                                 mybir.ActivationFunctionType.Exp, scale=-ALPHA)
            nc.vector.memset(g_all[:, 4, :], 1.0)

            for g in range(NG):
                p_grp = gpool.tile([128, GT, 3, COUT], F32, tag="pgrp")
                for tg in range(GT):
                    t = g * GT + tg
                    ps = ppool.tile([128, 4, 256], F32, tag="ps")
                    for i in range(3):
                        base = 64 + t * 128 + (i - 1) * 64
                        for k in range(KC):
                            nc.tensor.matmul(
                                ps[:, i, 0:192],
                                x_sb[:, k, base:base + 128],
                                w_sb[:, k, i, :],
                                start=(k == 0), stop=(k == KC - 1),
                            )
                    # tmp[p,i,j,o] = y3[p,i,(j,o)] * gate'[p,(i,j)]
                    tmp = wpool.tile([128, 3, 3, COUT], F32, tag="tmp")
                    gate_b = (g_all[:, :, t][:, :, None]
                              .broadcast_to((128, 9, COUT))
                              .rearrange("p (i j) o -> p i j o", i=3))
                    nc.vector.tensor_tensor(
                        tmp[:],
                        ps[:, 0:3, 0:192].rearrange("p i (j o) -> p i j o", j=3),
                        gate_b, op=mybir.AluOpType.mult)
                    # sum over i, keep (j, o)
                    nc.vector.tensor_reduce(
                        p_grp[:, tg, :, :],
                        tmp[:].rearrange("p i j o -> p j o i"),
                        axis=mybir.AxisListType.X, op=mybir.AluOpType.add)

                # DMA-accumulate the three j-streams into DRAM rows
                # row = 128*(8g+tg) + p + (2 - j); rows are pixel+1
                for j in range(3):
                    r0 = 1024 * g + 2 - j
                    dst = out_d[r0:r0 + 1024, :].rearrange(
                        "(t p) o -> p t o", p=128)
                    nc.gpsimd.dma_start(dst, p_grp[:, :, j, :],
                                        accum_op=mybir.AluOpType.add)

    nc.compile()
    return nc


def prep_inputs(input, depth, weight):
    """Host-side relayout: returns per-core in_maps."""
    # x: (B, 512, 64, 64) -> [128, KC, XCOLS] bf16 with zero guards
    xr = input.reshape(B, KC, 128, NPX).transpose(0, 2, 1, 3)  # [B,128,KC,NPX]
    x_all = np.zeros((B, 128, KC, XCOLS), dtype=ml_dtypes.bfloat16)
    x_all[:, :, :, 64:64 + NPX] = xr.astype(ml_dtypes.bfloat16)

    # w: (64, 512, 3, 3) -> [128, KC, 3(i), 192(j*64+o)] bf16
    wr = weight.reshape(COUT, KC, 128, 3, 3)
    w_dev = wr.transpose(2, 1, 3, 4, 0).reshape(128, KC, 3, 192)
    w_dev = np.ascontiguousarray(w_dev).astype(ml_dtypes.bfloat16)

    # gates are consumed at y-alignment q' (pre-shifted by 1-j):
    #   g'_ij[q'] = gate_ij at out pixel q = q' + 1 - j
    #   = exp(-a*|d[q] - d[q + off_ij]|),  off_ij = 64*(i-1) + (j-1)
    # dsh0 = d at out pixel q, dsh1 = d at neighbor; invalid -> dsh0+100
    d = depth.reshape(B, H, W).astype(np.float32)
    dflat = d.reshape(B, NPX)
    dsh0_all = np.empty((B, 128, 9, 32), dtype=np.float32)
    dsh1_all = np.empty((B, 128, 9, 32), dtype=np.float32)
    hh, ww = np.meshgrid(np.arange(H), np.arange(W), indexing="ij")
    qp = np.arange(NPX)
    for i in range(3):
        for j in range(3):
            # out pixel q = q' + 1 - j at y-alignment q'
            q = qp + 1 - j
            q_ok = (q >= 0) & (q < NPX)
            qc = np.clip(q, 0, NPX - 1)
            h_q, w_q = qc // W, qc % W
            # neighbor pixel (h+i-1, w+j-1) of out pixel q
            hn, wn = h_q + i - 1, w_q + j - 1
            n_ok = q_ok & (hn >= 0) & (hn < H) & (wn >= 0) & (wn < W)
            hnc = np.clip(hn, 0, H - 1)
            wnc = np.clip(wn, 0, W - 1)
            a = dflat[:, qc]                      # d at out pixel
            bV = d[:, hnc, wnc]                   # d at neighbor
            bV = np.where(n_ok[None, :], bV, a + 100.0)
            # [B, NPX] -> [B, p=(q'%128), t=(q'//128)] ; q' = h*64+w
            dsh0_all[:, :, 3 * i + j, :] = (
                a.reshape(B, 32, 128).transpose(0, 2, 1))
            dsh1_all[:, :, 3 * i + j, :] = (
                bV.reshape(B, 32, 128).transpose(0, 2, 1))

    return [
        {"x": x_all[b], "w": w_dev, "dsh0": dsh0_all[b], "dsh1": dsh1_all[b]}
        for b in range(B)
    ]


def kernel(input, depth, weight):
    input = np.asarray(input, dtype=np.float32)
    depth = np.asarray(depth, dtype=np.float32)
    weight = np.asarray(weight, dtype=np.float32)

    if "nc" not in _cache:
        _cache["nc"] = build_nc()
    nc = _cache["nc"]

    in_maps = prep_inputs(input, depth, weight)
    kwargs = {}
    if os.environ.get("KERNEL_TRACE") == "1":
        kwargs = dict(trace=True, trace_cores=list(range(B)))
    res = run_bass_kernel_spmd(nc, in_maps, core_ids=list(range(B)), **kwargs)
    _cache["last_results"] = res
    # out rows 1..4097 are pixel-major [NPX, COUT]
    out = np.stack([
        res.results[b]["out"][1:1 + NPX, :].T.reshape(COUT, H, W)
        for b in range(B)
    ]).astype(np.float32)
    return out


if __name__ == "__main__":
    rng = np.random.default_rng(0)
    x = rng.standard_normal((B, CIN, H, W), dtype=np.float32)
    d = rng.random((B, 1, H, W), dtype=np.float32)
    w = (rng.random((COUT, CIN, 3, 3), dtype=np.float32) - 0.5) * 0.08
    o = kernel(x, d, w)
    print(o.shape, o.dtype)
